# revision 16
# baseline (speedup 1.0000x reference)
"""Bahdanau-attention + reservoir-RNN cell fused Trainium2 kernel (fp8 rev).

Data-parallel over batch: B=128 split across 8 NeuronCores (16 rows each).
Weights replicated. Per core, for each batch row b:

    qT = Wa/bias matmuls on PE fp32 (q + Ua_b becomes the tanh bias)
    kT[o,s] = sum_h Ua[o,h]*x[s,h]   PE fp8e4 DoubleRow (K=256/pass), psum f32
    t = tanh(kT/64 + q[b])           ACT, psum -> t8 sbuf fp8 (Ua scaled by 64)
    scores = Va . t                  PE fp8 DoubleRow, Va scaled by 64
    e = exp(scores/64), l = sum(e)   ACT with accum_out (one FD=2048 call)
    context = (xT_bf16 . e) / l      DVE: bf16 mult (2x) + fused fold-adds
    h_next = tanh([x_t, ctx] @ WihT + h_prev @ WhhT + bias)   PE bf16 tail

The x tensor is DMA'd twice: fp8e4 (16MB/core) feeding the DoubleRow
matmuls and bf16 (32MB/core) feeding the context multiply (fp8 x would put
~2.5e-2 error on context; bf16 keeps it ~1e-3).

DMA plan (the previous rev was supply-starved: all 53MB through one HWDGE
ring -> head-of-line blocking between x8/xb slabs and pool-release waits
drained the pipeline every ~45us):
  - sync (SP HWDGE) ring:   q/k critical weights + all fp8 x slabs + hn out
  - scalar (ACT HWDGE) ring: h_next weights batch, issued at t=0, no waits
    (a waiting DMA on this ring would block ACTIVATEs behind it)
  - gpsimd (SWDGE) ring:    all bf16 x slabs + batched ctx out (the Pool
    sequencer is otherwise idle, so pool-release waits stall nothing)
Prefetch depth 4 on both slab streams.

context output is accumulated in SBUF ([P, HC, BPC] f32) and written with
ONE dma at the end: the old per-b scatter ([c p] -> p c) emitted 512
4-byte descriptors per row = 8448 tiny DMAs.

PSUM: one pool, 2 x (128, 2048) f32 tiles = all 8 banks. Request order
cycles warm/q/k(oc)/scores/h_next through the two buffers; Tile's
dependency tracking enforces drain-before-reuse.

Host-side sim of this exact quantization chain: h_next 7.2e-3,
context 1.28e-2 absmax-rel (tolerance 2e-2).
"""
import numpy as np
import ml_dtypes

import concourse.bacc as bacc
import concourse.tile as tile
from concourse import mybir
from concourse.bass_utils import run_bass_kernel_spmd

BF16 = ml_dtypes.bfloat16
E4M3 = ml_dtypes.float8_e4m3fn   # TRN fp8e4 matches OCP e4m3fn on +-0..240

B, S, E, H = 128, 2048, 512, 512
NCORES = 8
BPC = B // NCORES          # batch rows per core
P = 128
HC = H // P                # 4 chunks of 128 along H (and E)
EC = E // P                # 4
RC = EC + HC               # 8 contraction chunks for the rnn input
SC = 4                     # s-chunks of 512 (one PSUM bank each)
SCW = S // SC              # 512
WSCALE = 64.0              # host scale on Ua/Va before fp8 (avoids subnormals)
PRE = 4                    # slab prefetch depth (pool bufs)

_cache = {}


def _build():
    """Build the per-core Bass program (identical on all 8 cores)."""
    nc = bacc.Bacc("TRN2", target_bir_lowering=False, debug=False)
    f32, bf16 = mybir.dt.float32, mybir.dt.bfloat16
    fp8 = mybir.dt.float8e4

    # ALL inputs are pre-transposed on host into on-chip [partition, ...]
    # layout so every DMA is one large contiguous descriptor per partition
    # (512B-line weight rearranges starved the critical ring for ~30us:
    # SDMA round-robins between queues at PACKET granularity, so a queue
    # of tiny descriptors advances ~7KB per turn vs ~150KB for slab queues)
    x8_d = nc.dram_tensor("x8T", [BPC, P, HC, S], fp8, kind="ExternalInput")
    xb_d = nc.dram_tensor("xbT", [BPC, P, HC, S], bf16, kind="ExternalInput")
    ua8_d = nc.dram_tensor("ua8T", [P, HC, HC, P], fp8, kind="ExternalInput")
    va8_d = nc.dram_tensor("va8_rep", [P, HC, P], fp8, kind="ExternalInput")
    waT_d = nc.dram_tensor("waT", [P, HC, HC, P], bf16, kind="ExternalInput")
    qbias_d = nc.dram_tensor("qbias", [H], f32, kind="ExternalInput")     # Wa_b + Ua_b
    hpTb_d = nc.dram_tensor("hpTb", [P, HC, BPC], bf16, kind="ExternalInput")  # bf16 (q + Whh)
    xtTb_d = nc.dram_tensor("xtTb", [P, EC, BPC], bf16, kind="ExternalInput")
    wihT_d = nc.dram_tensor("wihT", [P, RC, H], bf16, kind="ExternalInput")
    whhT_d = nc.dram_tensor("whhT", [P, HC, H], bf16, kind="ExternalInput")
    hbias_d = nc.dram_tensor("hbias", [H], bf16, kind="ExternalInput")    # Wih_b + Whh_b

    hn_d = nc.dram_tensor("h_next", [BPC, H], f32, kind="ExternalOutput")
    # [p, hc, b] so the single output DMA is 256B-contiguous per partition
    ctx_d = nc.dram_tensor("context", [P, HC, BPC], f32, kind="ExternalOutput")

    with tile.TileContext(nc) as tc:
        with tc.tile_pool(name="weights", bufs=1) as wp, \
             tc.tile_pool(name="x8", bufs=PRE) as xp8, \
             tc.tile_pool(name="xb", bufs=PRE) as xpb, \
             tc.tile_pool(name="t8", bufs=2) as tp, \
             tc.tile_pool(name="e", bufs=2) as ep, \
             tc.tile_pool(name="scratch", bufs=1) as scp, \
             tc.tile_pool(name="small", bufs=2) as smp, \
             tc.tile_pool(name="psum", bufs=2, space="PSUM") as pp:

            # ---- ACT table warm (exp_and_others covers tanh+exp) ----
            warm_a = wp.tile([P, 8], f32)
            nc.vector.memset(warm_a[:], 0.0)
            warm_b = wp.tile([P, 8], f32)
            nc.scalar.activation(warm_b[:], warm_a[:],
                                 mybir.ActivationFunctionType.Tanh)

            # ---- PE HAM warmup overlapping the initial DMA phase ----
            warm_w = wp.tile([P, P], bf16)
            nc.vector.memset(warm_w[:], 0.125)
            warm_r = wp.tile([P, SCW], bf16)
            nc.vector.memset(warm_r[:], 0.5)
            warm_ps = pp.tile([P, S], f32, tag="ps")                       # psum req 0
            for _ in range(14):
                nc.tensor.matmul(warm_ps[:, 0:SCW], warm_w[:], warm_r[:],
                                 start=True, stop=True)

            # ---- startup DMAs: HWDGE rings are FIFO with ~2us completion
            # latency per DMA, so the critical weights are spread across all
            # three rings instead of queueing ahead of the first x slab ----
            # sync ring: wa (biggest q operand) + the fp8 slab stream
            wa_t = wp.tile([P, HC, HC, P], bf16)
            nc.sync.dma_start(wa_t[:], waT_d[:])
            # scalar ring: the k/scores fp8 weights (ACT is idle until the
            # first tanh, and these carry no waits so they can't block it)
            ua8_t = wp.tile([P, HC, HC, P], fp8)   # [hp, hc, oc, of]
            nc.scalar.dma_start(ua8_t[:], ua8_d[:])
            va8_t = wp.tile([P, HC, P], fp8)
            nc.scalar.dma_start(va8_t[:], va8_d[:])
            # SWDGE ring: small q operands, then the bf16 slab stream
            qbias_row = wp.tile([1, H], f32)
            nc.gpsimd.dma_start(qbias_row[:], qbias_d[:].rearrange("(one n) -> one n", one=1))
            hpb_t = wp.tile([P, HC, BPC], bf16)     # h_prev bf16: q matmul + Whh tail
            nc.gpsimd.dma_start(hpb_t[:], hpTb_d[:])
            ones_row = wp.tile([1, BPC], f32)
            nc.vector.memset(ones_row[:], 1.0)
            ones_rowb = wp.tile([1, BPC], bf16)
            nc.vector.memset(ones_rowb[:], 1.0)

            # h_next weights declared here, DMA'd one per iteration on the
            # SWDGE ring (deferring keeps startup HBM for the critical path)
            wih_t = wp.tile([P, RC, H], bf16)
            whh_t = wp.tile([P, HC, H], bf16)
            hbias_row = wp.tile([1, H], bf16)
            rnn_t = wp.tile([P, RC, BPC], bf16)

            # ---- slab preloads: fp8 on sync, bf16 on the SWDGE ring ----
            # (only 2 bf16 slabs upfront -- they're not needed until the
            # second loop iteration and would steal startup bandwidth)
            x8s, xbs = {}, {}
            for b in range(PRE):
                x8s[b] = xp8.tile([P, HC, S], fp8, tag="x8", name=f"x8_{b}")
                nc.sync.dma_start(x8s[b][:], x8_d[b])
            for b in range(2):
                xbs[b] = xpb.tile([P, HC, S], bf16, tag="xb", name=f"xb_{b}")
                nc.gpsimd.dma_start(xbs[b][:], xb_d[b])

            # ---- q phase: qb[o, b] = q[b, o] + Wa_b + Ua_b (the tanh bias) ----
            qps = pp.tile([P, S], f32, tag="ps")                           # psum req 1
            for oc in range(HC):
                sl = slice(oc * SCW, oc * SCW + BPC)
                for hc in range(HC):
                    nc.tensor.matmul(qps[:, sl], wa_t[:, hc, oc, :], hpb_t[:, hc, :],
                                     start=(hc == 0), stop=False)
                nc.tensor.matmul(qps[:, sl], qbias_row[0:1, oc * P:(oc + 1) * P],
                                 ones_row[:], start=False, stop=True)
            qb_t = wp.tile([P, HC, BPC], f32)
            nc.vector.tensor_copy(
                qb_t[:], qps[:].rearrange("p (c w) -> p c w", w=SCW)[:, :, 0:BPC])

            ctx_all = wp.tile([P, HC, BPC], f32)

            # ---- main loop, software-pipelined: the scores/exp/context phase
            # of batch b-1 is issued between oc1 and oc2 of batch b's k-phase,
            # so the PE never sits waiting on tanh(b-1, oc3) and the 2-buffer
            # PSUM ring always reuses a tile whose drain is >=2 stages back ----
            def k_phase(b, oc, x8_t, t8_t):
                kps = pp.tile([P, S], f32, tag="ps", name=f"kps_{b}_{oc}")
                for kp in range(2):
                    for sc in range(SC):
                        ssl = slice(sc * SCW, (sc + 1) * SCW)
                        nc.tensor.matmul(
                            kps[:, ssl],
                            ua8_t[:, 2 * kp:2 * kp + 2, oc, :],
                            x8_t[:, 2 * kp:2 * kp + 2, ssl],
                            start=(kp == 0), stop=(kp == 1),
                            perf_mode=mybir.MatmulPerfMode.DoubleRow)
                nc.scalar.activation(
                    t8_t[:, oc, :], kps[:],
                    mybir.ActivationFunctionType.Tanh,
                    bias=qb_t[:, oc, b:b + 1], scale=1.0 / WSCALE)

            def rest_phase(b, t8_t, xb_t):
                # scores: fp8 DoubleRow, Va(x64) replicated as stationary
                sps = pp.tile([P, S], f32, tag="ps", name=f"sps_{b}")
                for kp in range(2):
                    for sc in range(SC):
                        ssl = slice(sc * SCW, (sc + 1) * SCW)
                        nc.tensor.matmul(
                            sps[:, ssl],
                            va8_t[:, 2 * kp:2 * kp + 2, :],
                            t8_t[:, 2 * kp:2 * kp + 2, ssl],
                            start=(kp == 0), stop=(kp == 1),
                            perf_mode=mybir.MatmulPerfMode.DoubleRow)

                e_bc = ep.tile([P, S], bf16, tag="e")
                l1 = smp.tile([P, 1], f32, tag="l1")
                nc.scalar.activation(e_bc[:], sps[:],
                                     mybir.ActivationFunctionType.Exp,
                                     scale=1.0 / WSCALE, accum_out=l1[:])
                rl = smp.tile([P, 1], f32, tag="rl")
                nc.vector.reciprocal(rl[:], l1[:])

                # context: one bf16 mult (2x) with e broadcast over the hc
                # dim (0-stride AP) + fused fold-adds over all hc at once
                # (fewer DVE instruction bubbles than per-hc chains),
                # folding down to FD=512 before the 1x-rate tensor_reduce
                scr4 = scp.tile([P, HC, S], bf16, tag="scr")
                e_b4 = e_bc[:].rearrange("p (one s) -> p one s", one=1) \
                              .broadcast_to([P, HC, S])
                nc.vector.tensor_tensor(scr4[:], xb_t[:], e_b4,
                                        mybir.AluOpType.mult)
                g1 = scp.tile([P, HC, S // 2], bf16, tag="g1")
                nc.vector.tensor_tensor(g1[:], scr4[:, :, 0:S // 2],
                                        scr4[:, :, S // 2:S], mybir.AluOpType.add)
                g2 = scp.tile([P, HC, S // 4], bf16, tag="g2")
                nc.vector.tensor_tensor(g2[:], g1[:, :, 0:S // 4],
                                        g1[:, :, S // 4:S // 2], mybir.AluOpType.add)
                g3 = scp.tile([P, HC, S // 8], bf16, tag="g3")
                nc.vector.tensor_tensor(g3[:], g2[:, :, 0:S // 8],
                                        g2[:, :, S // 8:S // 4], mybir.AluOpType.add)
                g4 = scp.tile([P, HC, S // 16], bf16, tag="g4")
                nc.vector.tensor_tensor(g4[:], g3[:, :, 0:S // 16],
                                        g3[:, :, S // 16:S // 8], mybir.AluOpType.add)
                ctx_sb = smp.tile([P, HC], f32, tag="ctx")
                nc.vector.tensor_reduce(ctx_sb[:], g4[:], mybir.AxisListType.X,
                                        mybir.AluOpType.add)
                nc.vector.tensor_scalar_mul(ctx_all[:, :, b], ctx_sb[:], rl[:])
                nc.vector.tensor_copy(rnn_t[:, EC:RC, b:b + 1],
                                      ctx_all[:, :, b:b + 1])

            t8s = {}
            for b in range(BPC):
                if b + PRE < BPC:
                    x8s[b + PRE] = xp8.tile([P, HC, S], fp8, tag="x8",
                                            name=f"x8_{b + PRE}")
                    nc.sync.dma_start(x8s[b + PRE][:], x8_d[b + PRE])
                if 2 <= b + 2 < BPC:
                    xbs[b + 2] = xpb.tile([P, HC, S], bf16, tag="xb",
                                          name=f"xb_{b + 2}")
                    nc.gpsimd.dma_start(xbs[b + 2][:], xb_d[b + 2])
                t8s[b] = tp.tile([P, HC, S], fp8, tag="t8", name=f"t8_{b}")
                k_phase(b, 0, x8s[b], t8s[b])
                k_phase(b, 1, x8s[b], t8s[b])
                if b > 0:
                    rest_phase(b - 1, t8s[b - 1], xbs[b - 1])
                    del t8s[b - 1], xbs[b - 1], x8s[b - 1]
                k_phase(b, 2, x8s[b], t8s[b])
                k_phase(b, 3, x8s[b], t8s[b])
                # deferred h_next weight loads, one per iteration (SWDGE)
                if b == 0:
                    nc.gpsimd.dma_start(wih_t[:], wihT_d[:])
                elif b == 1:
                    nc.gpsimd.dma_start(whh_t[:], whhT_d[:])
                elif b == 2:
                    nc.gpsimd.dma_start(
                        hbias_row[:], hbias_d[:].rearrange("(one n) -> one n", one=1))
                elif b == 3:
                    nc.gpsimd.dma_start(rnn_t[:, 0:EC, :], xtTb_d[:])
            rest_phase(BPC - 1, t8s[BPC - 1], xbs[BPC - 1])

            # single batched context store (SWDGE ring, idle by now)
            nc.gpsimd.dma_start(ctx_d[:], ctx_all[:])

            # ---- h_next tail: all bf16 matmuls + tanh ----
            # the final rest-phase DVE chain leaves the PE idle for ~15us;
            # burn dummy matmuls into an unused psum region so HAM doesn't
            # re-throttle and the h_next matmuls run at full clock
            hnp = pp.tile([P, S], f32, tag="ps")
            for _ in range(24):
                nc.tensor.matmul(hnp[:, SCW:2 * SCW], warm_w[:], warm_r[:],
                                 start=True, stop=True)
            for c in range(EC):
                nc.tensor.matmul(hnp[0:BPC, 0:H], rnn_t[:, c, :], wih_t[:, c, :],
                                 start=(c == 0), stop=False)
            for hc in range(HC):
                nc.tensor.matmul(hnp[0:BPC, 0:H], hpb_t[:, hc, :], whh_t[:, hc, :],
                                 start=False, stop=False)
            nc.tensor.matmul(hnp[0:BPC, 0:H], ones_rowb[:], hbias_row[:],
                             start=False, stop=False)
            for c in range(EC, RC):
                nc.tensor.matmul(hnp[0:BPC, 0:H], rnn_t[:, c, :], wih_t[:, c, :],
                                 start=False, stop=(c == RC - 1))
            hn_sb = smp.tile([BPC, H], f32, tag="hn")
            nc.scalar.activation(hn_sb[:], hnp[0:BPC, 0:H],
                                 mybir.ActivationFunctionType.Tanh)
            nc.sync.dma_start(hn_d[:], hn_sb[:])

    nc.compile()
    return nc


def _prep_host_inputs(x_t, x_ref_encoded, h_prev, Wa_w, Wa_b, Ua_w, Ua_b,
                      Va_w, Va_b, Wih_w, Wih_b, Whh_w, Whh_b):
    """Shard + transpose + cast on host. Returns in_maps for the 8 cores."""
    def q8(a):
        return np.clip(a, -240.0, 240.0).astype(E4M3)

    def part_major(w2d):
        # [K, N] with K = kc*128 + kp  ->  [kp, kc, N] (contiguous per partition)
        kc = w2d.shape[0] // P
        return np.ascontiguousarray(w2d.reshape(kc, P, -1).transpose(1, 0, 2))

    # [hp, hc, oc, of] = W.T[hc*128+hp, oc*128+of]
    ua8T = np.ascontiguousarray(
        q8(Ua_w.T * WSCALE).reshape(HC, P, HC, P).transpose(1, 0, 2, 3))
    waT = np.ascontiguousarray(
        Wa_w.T.reshape(HC, P, HC, P).transpose(1, 0, 2, 3)).astype(BF16)
    wihT = part_major(Wih_w.T.astype(BF16))
    whhT = part_major(Whh_w.T.astype(BF16))
    va8_rep = np.ascontiguousarray(
        np.broadcast_to(q8(Va_w[0] * WSCALE).reshape(HC, P, 1),
                        (HC, P, P)).transpose(1, 0, 2))
    qbias = (Wa_b + Ua_b)
    hbias = (Wih_b + Whh_b).astype(BF16)

    x_bf = x_ref_encoded.astype(BF16)
    x_f8 = q8(x_ref_encoded)
    in_maps = []
    for c in range(NCORES):
        rows = slice(c * BPC, (c + 1) * BPC)
        # [b, p, hc, s]: per-partition contiguous for single-descriptor DMA
        xbT = np.ascontiguousarray(
            x_bf[rows].reshape(BPC, S, HC, P).transpose(0, 3, 2, 1))
        x8T = np.ascontiguousarray(
            x_f8[rows].reshape(BPC, S, HC, P).transpose(0, 3, 2, 1))
        hpTb = part_major(h_prev[rows].T.astype(BF16))
        xtTb = part_major(x_t[rows, 0, :].T.astype(BF16))
        in_maps.append({
            "x8T": x8T, "xbT": xbT, "ua8T": ua8T, "waT": waT, "va8_rep": va8_rep,
            "qbias": qbias, "hpTb": hpTb, "xtTb": xtTb,
            "wihT": wihT, "whhT": whhT, "hbias": hbias,
        })
    return in_maps


def kernel(x_t, x_ref_encoded, h_prev, Wa_w, Wa_b, Ua_w, Ua_b, Va_w, Va_b,
           Wih_w, Wih_b, Whh_w, Whh_b, _trace=False, _tmpdir=None):
    if "nc" not in _cache:
        _cache["nc"] = _build()
    nc = _cache["nc"]

    in_maps = _prep_host_inputs(
        np.asarray(x_t), np.asarray(x_ref_encoded), np.asarray(h_prev),
        np.asarray(Wa_w), np.asarray(Wa_b), np.asarray(Ua_w), np.asarray(Ua_b),
        np.asarray(Va_w), np.asarray(Va_b), np.asarray(Wih_w), np.asarray(Wih_b),
        np.asarray(Whh_w), np.asarray(Whh_b))

    res = run_bass_kernel_spmd(nc, in_maps, core_ids=list(range(NCORES)),
                               trace=_trace, tmpdir=_tmpdir)
    _cache["last_result"] = res

    h_next = np.concatenate([res.results[c]["h_next"] for c in range(NCORES)], axis=0)
    # context arrives as [p, hc, b] per core -> [b, hc*128+p]
    context = np.concatenate(
        [np.ascontiguousarray(res.results[c]["context"].transpose(2, 1, 0))
         .reshape(BPC, H) for c in range(NCORES)], axis=0)
    return (h_next, context)
